# revision 15
# baseline (speedup 1.0000x reference)
"""MoE (top-2 of 8 experts) forward on 8 Trainium2 NeuronCores.

Strategy (expert parallel, collective-free):
  - core c owns expert c (w1[c], w2[c] are the only sharded inputs, bf16).
  - every core computes the full routing on device from a host-pretransposed
    xT (fp32 logits via 24 wide matmuls, pipelined per 512-token chunk;
    top-2 via DVE max/max_index; expert-c counting sort via one
    strict-triangular prefix matmul + two tiny transpose matmuls).
  - the core's compact token list (capacity C=640 >= observed max 527) is
    materialized on the tensor engine: one-hot slot masks (DVE is_equal)
    select each copy's (tok_hi, tok_lo, weight) payload into list[slot]
    via accumulating matmuls -- no DRAM scatter round trip.
  - the compact MLP (two grouped GEMMs + silu, bf16 operands / fp32 psum)
    runs batched over all C tokens: GEMM1 streams 640-wide activations per
    (f,h) chunk; GEMM2 accumulates y per 128-token tile over all 24 f chunks.
  - each core writes its compact outputs [C, H] bf16 (routing weight already
    applied) + the (token, weight) list; the host scatter-adds rows into the
    full [2048, 768] output.  No collectives -> no inter-core barrier, no
    start-skew coupling, no all-to-all tail.

kernel(**inputs) -> full [2048, 768] float32 output.
"""
import sys

sys.path.insert(0, "/opt/trn_rl_repo")

import numpy as np

import concourse.bass as bass
import concourse.mybir as mybir
import concourse.tile as tile
from concourse.bass import IndirectOffsetOnAxis

F32 = mybir.dt.float32
BF16 = mybir.dt.bfloat16
I32 = mybir.dt.int32
U32 = mybir.dt.uint32
AF = mybir.ActivationFunctionType
OP = mybir.AluOpType
AX = mybir.AxisListType

T, H, E, K, F = 2048, 768, 8, 2, 3072
P = 128
NCORE = 8
NT = T // P          # 16 token tiles
NH = H // P          # 6 hidden chunks
NF = F // P          # 24 ffn chunks
C = 640              # compact-list capacity per expert (mean 512, obs max 527)
NC = C // P          # 5 compact tiles
BIG = 8192.0

# packed f32 constant table columns
PK_U = 0          # [P, P] strict upper triangular
PK_IOTA = 128     # [P, C] slot iota
PK_THI = 768      # [P, NT] token>>8
PK_TLO = 784      # [P, NT] token&255
PK_ME = 800       # [P, 1] my expert id
PK_W = 808        # total (padded)

# ---------------------------------------------------------------------------
# This container's walrus cannot attach sem-wait commands to most
# instruction types. Two workarounds (see _split_attached_waits and the
# patched kernel-tail below): waits are moved onto standalone
# EventSemaphore instructions, and the Tile tail drain's waits are
# split across a chain of SP nops.
_MAX_WAITS = 4


def _patched_drain_and_barrier(self, tick_clock, wait_clock):
    from concourse.tile import ScopedClock, VectorClock
    from concourse.tile_sem_assignment import N_PROCS

    g = tick_clock.global_clock
    ticks = [g[p] for p in range(N_PROCS)]
    procs = [p for p in range(N_PROCS) if ticks[p] > 0]
    observed = [0] * N_PROCS
    for i in range(0, len(procs), _MAX_WAITS):
        chunk = set(procs[i : i + _MAX_WAITS])
        part = VectorClock([ticks[p] if p in chunk else 0 for p in range(N_PROCS)])
        nop = self.nc.sync.nop()
        wait_clock.add_sem_waits(
            nop.ins,
            ScopedClock({None: part}),
            ScopedClock({None: VectorClock(list(observed))}),
        )
        for p in chunk:
            observed[p] = ticks[p]
    drain_inst = self.nc.sync.drain()
    wait_clock.add_sem_waits(
        drain_inst.ins,
        ScopedClock({None: g}),
        ScopedClock({None: VectorClock(list(observed))}),
    )
    self.nc.all_engine_barrier()
    assert self.sems is not None
    popped = self.nc._tile_sem_poison_stack.pop()
    assert popped is self._sem_poison
    self.nc.clear_and_free_semaphores(list(self.sems.allocated().values()))
    self.nc.all_engine_barrier()


tile.TileContext._drain_and_barrier = _patched_drain_and_barrier


def _split_attached_waits(nc):
    n = 0
    for f in nc.m.functions:
        for bb in f.blocks:
            new = []
            for inst in bb.instructions:
                si = getattr(inst, "sync_info", None)
                waits = list(si.on_wait) if (si and si.on_wait) else []
                if waits and not isinstance(inst, mybir.InstEventSemaphore):
                    for k, w in enumerate(waits):
                        n += 1
                        new.append(
                            mybir.InstEventSemaphore(
                                name=f"{inst.name}-w{k}",
                                engine=inst.engine,
                                ins=[],
                                outs=[],
                                sync_info=mybir.SyncInfo(on_wait=[w], on_update=[]),
                            )
                        )
                    si.on_wait = []
                new.append(inst)
            bb.instructions[:] = new
    return n


def build_nc():
    nc = bass.Bass(num_devices=NCORE)
    xt_d = nc.declare_dram_parameter("xT", [H, T], F32, isOutput=False)
    xb_d = nc.declare_dram_parameter("xb", [T, H], BF16, isOutput=False)
    rw_d = nc.declare_dram_parameter("rw", [H, E], F32, isOutput=False)
    w1_d = nc.declare_dram_parameter("w1c", [H, F], BF16, isOutput=False)
    w2_d = nc.declare_dram_parameter("w2c", [F, H], BF16, isOutput=False)
    id_d = nc.declare_dram_parameter("identb", [P, P], BF16, isOutput=False)
    ii_d = nc.declare_dram_parameter("ii", [16, 24], F32, isOutput=False)
    pk_d = nc.declare_dram_parameter("pk", [P, PK_W], F32, isOutput=False)
    outy_d = nc.declare_dram_parameter("out_y", [C, H], BF16, isOutput=True)
    outm_d = nc.declare_dram_parameter("out_m", [C, 2], F32, isOutput=True)

    tc = tile.TileContext(nc)
    with tc:
        with (
            tc.tile_pool(name="consts", bufs=1) as cb,
            tc.tile_pool(name="weights", bufs=1) as wp,
            tc.tile_pool(name="work", bufs=2) as wk,
        ):
            # ---- input DMAs, latency-critical first ----
            rw_t = []
            for h in range(NH):
                t = cb.tile([P, E], F32, tag=f"rw{h}", name=f"rw{h}")
                nc.sync.dma_start(t, rw_d[P * h : P * (h + 1), :])
                rw_t.append(t)
            xt_t = [[None] * 4 for _ in range(NH)]
            for q in range(4):
                for h in range(NH):
                    t = wk.tile([P, 512], F32, tag=f"xt{h}_{q}", bufs=1,
                                name=f"xt{h}_{q}")
                    nc.sync.dma_start(
                        t, xt_d[P * h : P * (h + 1), 512 * q : 512 * (q + 1)]
                    )
                    xt_t[h][q] = t
            ii = cb.tile([16, 24], F32, tag="ii")
            nc.sync.dma_start(ii, ii_d[:, :])
            pk = cb.tile([P, PK_W], F32, tag="pk")
            nc.sync.dma_start(pk, pk_d[:, :])
            ident_bf = cb.tile([P, P], BF16, tag="ident_bf")
            nc.sync.dma_start(ident_bf, id_d[:, :])
            # weights: issue from otherwise-idle engine queues
            w1_t = []
            for h in range(NH):
                t = wp.tile([P, F], BF16, tag=f"w1_{h}", name=f"w1_{h}")
                nc.scalar.dma_start(t, w1_d[P * h : P * (h + 1), :])
                w1_t.append(t)
            w2_t = []
            for f in range(NF):
                t = wp.tile([P, H], BF16, tag=f"w2_{f}", name=f"w2_{f}")
                nc.gpsimd.dma_start(t, w2_d[P * f : P * (f + 1), :])
                w2_t.append(t)
            U = pk[:, PK_U : PK_U + P]
            iota = pk[:, PK_IOTA : PK_IOTA + C]
            i8 = ii[0:8, 0:8]
            u16 = ii[0:16, 8:24]
            ones_row = cb.tile([1, P], F32, tag="ones_row")
            nc.vector.memset(ones_row, 1.0)
            ones_col = cb.tile([P, 1], F32, tag="ones_col")
            nc.vector.memset(ones_col, 1.0)

            # ---- logitsT = rw.T @ xT (fp32), pipelined per 512-token chunk:
            # transpose + top-2 of chunk q overlap the matmuls of chunk q+1
            lgT_sb = cb.tile([8, T], F32, tag="lgT_sb")
            lgA = cb.tile([P, NT, E], F32, tag="lgA")
            valsA = cb.tile([P, NT, 8], F32, tag="valsA")
            idxA = cb.tile([P, NT, 8], U32, tag="idxA")
            with tc.tile_pool(name="psr", bufs=1, space="PSUM") as pr:
                lgT_ps = pr.tile([8, T], F32, tag="lgT", space="PSUM")
                for q in range(4):
                    for h in range(NH):
                        nc.tensor.matmul(
                            lgT_ps[:, 512 * q : 512 * (q + 1)],
                            lhsT=rw_t[h],
                            rhs=xt_t[h][q],
                            start=(h == 0),
                            stop=(h == NH - 1),
                        )
                    nc.vector.tensor_copy(
                        lgT_sb[:, 512 * q : 512 * (q + 1)],
                        lgT_ps[:, 512 * q : 512 * (q + 1)],
                    )
                    for i in range(4 * q, 4 * q + 4):
                        tp = pr.tile([P, 8], F32, tag="tps", bufs=4, space="PSUM")
                        nc.tensor.matmul(
                            tp,
                            lhsT=lgT_sb[:, P * i : P * (i + 1)],
                            rhs=i8,
                            start=True,
                            stop=True,
                        )
                        nc.scalar.copy(lgA[:, i, :], tp)
                        nc.vector.max(out=valsA[:, i, :], in_=lgA[:, i, :])
                        nc.vector.max_index(
                            out=idxA[:, i, :],
                            in_max=valsA[:, i, :],
                            in_values=lgA[:, i, :],
                        )

            # ---- top-2 weights + expert-c membership masks ----
            idxf2 = cb.tile([P, NT, 2], F32, tag="idxf2")
            nc.vector.tensor_copy(idxf2, idxA[:, :, 0:2])
            m1A = wk.tile([P, NT], F32, tag="m1A")
            nc.vector.tensor_tensor(
                out=m1A,
                in0=idxf2[:, :, 0],
                in1=pk[:, PK_ME : PK_ME + 1].to_broadcast([P, NT]),
                op=OP.is_equal,
            )
            m2A = wk.tile([P, NT], F32, tag="m2A")
            nc.vector.tensor_tensor(
                out=m2A,
                in0=idxf2[:, :, 1],
                in1=pk[:, PK_ME : PK_ME + 1].to_broadcast([P, NT]),
                op=OP.is_equal,
            )
            m_cA = wk.tile([P, NT], F32, tag="m_cA")
            nc.vector.tensor_tensor(out=m_cA, in0=m1A, in1=m2A, op=OP.add)
            dA = wk.tile([P, NT], F32, tag="dA")
            nc.vector.tensor_tensor(
                out=dA, in0=valsA[:, :, 1], in1=valsA[:, :, 0], op=OP.subtract
            )
            eA = wk.tile([P, NT], F32, tag="eA")
            nc.scalar.activation(out=eA, in_=dA, func=AF.Exp)
            smA = wk.tile([P, NT], F32, tag="smA")
            nc.vector.tensor_scalar_add(smA, eA, 1.0)
            w1nA = wk.tile([P, NT], F32, tag="w1nA")
            nc.vector.reciprocal(w1nA, smA)
            t1A = wk.tile([P, NT], F32, tag="t1A")
            nc.vector.tensor_tensor(out=t1A, in0=m1A, in1=w1nA, op=OP.mult)
            w2nA = wk.tile([P, NT], F32, tag="w2nA")
            nc.vector.tensor_tensor(out=w2nA, in0=eA, in1=w1nA, op=OP.mult)
            t2A = wk.tile([P, NT], F32, tag="t2A")
            nc.vector.tensor_tensor(out=t2A, in0=m2A, in1=w2nA, op=OP.mult)
            w_cA = wk.tile([P, NT], F32, tag="w_cA")
            nc.vector.tensor_tensor(out=w_cA, in0=t1A, in1=t2A, op=OP.add)

            # ---- counting sort for expert c only:
            # slot(t) = strict-prefix(m_c)[t] + base[tile(t)]
            laccT = cb.tile([P, NC, 3], F32, tag="laccT")
            tok_fA = cb.tile([P, NC], F32, tag="tok_fA")
            xsT = cb.tile([P, NH, C], BF16, tag="xsT")
            sel_all = cb.tile([P, NT, C], BF16, tag="sel_all")
            with tc.tile_pool(name="ps2", bufs=1, space="PSUM") as p2:
                cntT_ps = p2.tile([NT, 1], F32, tag="cntT", space="PSUM")
                nc.tensor.matmul(
                    cntT_ps, lhsT=m_cA, rhs=ones_col, start=True, stop=True
                )
                cntT_sb = cb.tile([NT, 1], F32, tag="cntT_sb")
                nc.vector.tensor_copy(cntT_sb, cntT_ps)
                base_ps = p2.tile([1, NT], F32, tag="base", space="PSUM")
                nc.tensor.matmul(
                    base_ps, lhsT=cntT_sb, rhs=u16, start=True, stop=True
                )
                base_sb = cb.tile([1, NT], F32, tag="base_sb")
                nc.vector.tensor_copy(base_sb, base_ps)
                slot_ps = p2.tile([P, NT], F32, tag="slot", space="PSUM")
                nc.tensor.matmul(slot_ps, lhsT=U, rhs=m_cA, start=True, stop=False)
                nc.tensor.matmul(
                    slot_ps, lhsT=ones_row, rhs=base_sb, start=False, stop=True
                )
                slot_cA = wk.tile([P, NT], F32, tag="slot_cA")
                nc.vector.tensor_copy(slot_cA, slot_ps)
                nmA = wk.tile([P, NT], F32, tag="nmA")
                nc.vector.tensor_scalar(nmA, m_cA, -BIG, BIG, op0=OP.mult, op1=OP.add)
                slot_mA = wk.tile([P, NT], F32, tag="slot_mA")
                nc.vector.tensor_tensor(out=slot_mA, in0=slot_cA, in1=nmA, op=OP.add)
                payloadA = wk.tile([P, NT, 3], BF16, tag="payloadA")
                nc.vector.tensor_copy(payloadA[:, :, 0], pk[:, PK_THI : PK_THI + NT])
                nc.vector.tensor_copy(payloadA[:, :, 1], pk[:, PK_TLO : PK_TLO + NT])
                nc.vector.tensor_copy(payloadA[:, :, 2], w_cA)
                for k in range(2):
                    nc.vector.tensor_tensor(
                        out=sel_all[:, 8 * k : 8 * (k + 1), :],
                        in0=slot_mA[:, 8 * k : 8 * (k + 1)]
                        .unsqueeze(2)
                        .to_broadcast([P, 8, C]),
                        in1=iota.unsqueeze(1).to_broadcast([P, 8, C]),
                        op=OP.is_equal,
                    )

            # ---- compact list via one-hot selection matmuls (no DRAM trip):
            # list[c, s] = sum_t payload[t, c] * [slot(t) == s]
            with tc.tile_pool(name="psL", bufs=1, space="PSUM") as pL:
                list_ps = pL.tile([3, C], F32, tag="list", space="PSUM")
                for i in range(NT):
                    nc.tensor.matmul(
                        list_ps[:, 0:512],
                        lhsT=payloadA[:, i, :],
                        rhs=sel_all[:, i, 0:512],
                        start=(i == 0),
                        stop=(i == NT - 1),
                    )
                    nc.tensor.matmul(
                        list_ps[:, 512:C],
                        lhsT=payloadA[:, i, :],
                        rhs=sel_all[:, i, 512:C],
                        start=(i == 0),
                        stop=(i == NT - 1),
                    )
                # keep the PE HAM busy window open through the gather phase
                # (idle > ~3.4us would halve the PE clock entering the MLP)
                for w in range(12):
                    warm_ps = pL.tile([8, P], F32, tag="tpl", bufs=2, space="PSUM")
                    nc.tensor.matmul(
                        warm_ps, lhsT=rw_t[0], rhs=U, start=True, stop=True
                    )
                list_sb = cb.tile([3, C], F32, tag="list_sb")
                nc.vector.tensor_copy(list_sb, list_ps)
                # per-tile pipeline: transpose list -> token idx -> gather ->
                # transpose gathered rows (gather j overlaps transposes j-1)
                for j in range(NC):
                    tpl = pL.tile([P, 3], F32, tag="tpl", bufs=2, space="PSUM")
                    nc.tensor.matmul(
                        tpl,
                        lhsT=list_sb[:, P * j : P * (j + 1)],
                        rhs=ii[0:3, 0:3],
                        start=True,
                        stop=True,
                    )
                    nc.vector.tensor_copy(laccT[:, j, :], tpl)
                    nc.vector.tensor_scalar(
                        tok_fA[:, j : j + 1], laccT[:, j, 0:1], 256.0, None,
                        op0=OP.mult,
                    )
                    nc.vector.tensor_tensor(
                        out=tok_fA[:, j : j + 1],
                        in0=tok_fA[:, j : j + 1],
                        in1=laccT[:, j, 1:2],
                        op=OP.add,
                    )
                    idx_j = wk.tile([P, 1], I32, tag="idx_j")
                    nc.vector.tensor_copy(idx_j, tok_fA[:, j : j + 1])
                    xs = wk.tile([P, H], BF16, tag="xs", bufs=3)
                    nc.gpsimd.indirect_dma_start(
                        out=xs[:, :],
                        out_offset=None,
                        in_=xb_d[:, :],
                        in_offset=IndirectOffsetOnAxis(ap=idx_j[:, 0:1], axis=0),
                        bounds_check=T - 1,
                        oob_is_err=False,
                    )
                    for h in range(NH):
                        tp = pL.tile([P, P], F32, tag="tps", bufs=4, space="PSUM")
                        nc.tensor.matmul(
                            tp,
                            lhsT=xs[:, P * h : P * (h + 1)],
                            rhs=ident_bf,
                            start=True,
                            stop=True,
                        )
                        nc.scalar.copy(xsT[:, h, P * j : P * (j + 1)], tp)

            # ---- GEMM1: h = silu(w1.T @ xsT), batched over all C tokens ----
            h_all = cb.tile([P, NF, C], BF16, tag="h_all")
            with tc.tile_pool(name="ps4", bufs=1, space="PSUM") as p4:
                for f in range(NF):
                    psA = p4.tile([P, 512], F32, tag="psA", bufs=2, space="PSUM")
                    psB = p4.tile([P, C - 512], F32, tag="psB", bufs=2, space="PSUM")
                    for h in range(NH):
                        lw = w1_t[h][:, P * f : P * (f + 1)]
                        nc.tensor.matmul(
                            psA,
                            lhsT=lw,
                            rhs=xsT[:, h, 0:512],
                            start=(h == 0),
                            stop=(h == NH - 1),
                        )
                        nc.tensor.matmul(
                            psB,
                            lhsT=lw,
                            rhs=xsT[:, h, 512:C],
                            start=(h == 0),
                            stop=(h == NH - 1),
                        )
                    nc.scalar.activation(
                        out=h_all[:, f, 0:512], in_=psA, func=AF.Silu
                    )
                    nc.scalar.activation(
                        out=h_all[:, f, 512:C], in_=psB, func=AF.Silu
                    )

                # ---- GEMM2: y = h.T @ w2 per token tile; scale; write out ----
                for j in range(NC):
                    y_ps = p4.tile([P, H], F32, tag="yps", bufs=2, space="PSUM")
                    for f in range(NF):
                        lh = h_all[:, f, P * j : P * (j + 1)]
                        nc.tensor.matmul(
                            y_ps[:, 0:512],
                            lhsT=lh,
                            rhs=w2_t[f][:, 0:512],
                            start=(f == 0),
                            stop=(f == NF - 1),
                        )
                        nc.tensor.matmul(
                            y_ps[:, 512:H],
                            lhsT=lh,
                            rhs=w2_t[f][:, 512:H],
                            start=(f == 0),
                            stop=(f == NF - 1),
                        )
                    y_sb = wk.tile([P, H], BF16, tag="y_sb")
                    nc.vector.tensor_scalar(
                        y_sb, y_ps, laccT[:, j, 2:3], None, op0=OP.mult
                    )
                    nc.sync.dma_start(outy_d[P * j : P * (j + 1), :], y_sb)
            outm_sb = cb.tile([P, NC, 2], F32, tag="outm_sb")
            nc.vector.tensor_copy(outm_sb[:, :, 0], tok_fA)
            nc.vector.tensor_copy(outm_sb[:, :, 1], laccT[:, :, 2])
            nc.sync.dma_start(outm_d.rearrange("(a p) c -> p a c", p=P), outm_sb)

    _split_attached_waits(nc)
    return nc


def make_in_maps(x, router_w, w1, w2):
    import ml_dtypes

    bf16 = ml_dtypes.bfloat16
    x = np.ascontiguousarray(np.asarray(x, np.float32))
    rw = np.ascontiguousarray(np.asarray(router_w, np.float32))
    w1 = np.asarray(w1, np.float32)
    w2 = np.asarray(w2, np.float32)

    xT = np.ascontiguousarray(x.T)
    xb = np.ascontiguousarray(x.astype(bf16))
    identb = np.eye(P, dtype=np.float32).astype(bf16)
    ii = np.zeros((16, 24), np.float32)
    ii[0:8, 0:8] = np.eye(8)
    ii[:, 8:24] = np.triu(np.ones((16, 16), np.float32), 1)
    tokA = np.arange(P)[:, None] + P * np.arange(NT)[None, :]
    pk = np.zeros((P, PK_W), np.float32)
    pk[:, PK_U : PK_U + P] = np.triu(np.ones((P, P), np.float32), 1)
    pk[:, PK_IOTA : PK_IOTA + C] = np.arange(C, dtype=np.float32)[None, :]
    pk[:, PK_THI : PK_THI + NT] = tokA // 256
    pk[:, PK_TLO : PK_TLO + NT] = tokA % 256
    in_maps = []
    for c in range(NCORE):
        pkc = pk.copy()
        pkc[:, PK_ME] = float(c)
        in_maps.append(
            {
                "xT": xT,
                "xb": xb,
                "rw": rw,
                "w1c": np.ascontiguousarray(w1[c].astype(bf16)),
                "w2c": np.ascontiguousarray(w2[c].astype(bf16)),
                "identb": identb,
                "ii": ii,
                "pk": pkc,
            }
        )
    return in_maps


def gather_output(results):
    out = np.zeros((T, H), np.float32)
    for c in range(NCORE):
        y = np.asarray(results[c]["out_y"], np.float32)
        m = np.asarray(results[c]["out_m"], np.float32)
        tok = m[:, 0].astype(np.int64)
        np.add.at(out, tok, y)
    return out


def kernel(x, router_w, w1, w2):
    from concourse.bass_utils import run_bass_kernel_spmd

    nc = build_nc()
    in_maps = make_in_maps(x, router_w, w1, w2)
    res = run_bass_kernel_spmd(nc, in_maps, list(range(NCORE)))
    return gather_output(res.results)
